# revision 16
# baseline (speedup 1.0000x reference)
"""DGCNN part-segmentation forward on 8 TRN2 NeuronCores (Bass/Tile).

Data parallel: B=16 clouds -> 8 cores x 2 clouds each. Per cloud:
  STN -> rotate -> 3x [pairwise-distance matmul -> exact top-20 on DVE
  (max8 / max_index / match_replace) -> gpsimd ap_gather of neighbor
  features (per-core index sets, DRAM-bounce rewrap) -> edge convs as
  accumulating matmuls with a broadcast center term] -> global-max conv6
  -> conv8/9/10/11 tail -> tanh.  All fp32.
"""
import os
import numpy as np

import concourse.bass as bass
import concourse.mybir as mybir
import concourse.tile as tile
import concourse.bacc as bacc
from concourse.bass_utils import run_bass_kernel_spmd

F32 = mybir.dt.float32
U16 = mybir.dt.uint16
I16 = mybir.dt.int16
AF = mybir.ActivationFunctionType
ALU = mybir.AluOpType
AX = mybir.AxisListType

N = 2048
K = 20
TRUNC = int(os.environ.get("KERNEL_TRUNC", "0"))
NT = N // 128
BPC = 2
NEG = -1e30

_cache = {}


def _lrelu_np(v):
    return np.where(v > 0, v, np.float32(0.2) * v).astype(np.float32)


def _prep_weights(p):
    g = lambda k: np.asarray(p[k], np.float32)
    w = {}
    w["c1T"] = np.ascontiguousarray(g("stn_c1_w").T)
    w["c2T"] = np.ascontiguousarray(g("stn_c2_w").T)
    w["c3T"] = np.ascontiguousarray(g("stn_c3_w").T)
    fc1T = g("stn_fc1_w").T
    w["fc1T"] = np.concatenate([fc1T[128 * k:128 * (k + 1)] for k in range(8)], axis=1)
    fc2T = g("stn_fc2_w").T
    w["fc2T"] = np.concatenate([fc2T[128 * k:128 * (k + 1)] for k in range(4)], axis=1)
    fc3T = g("stn_fc3_w").T
    w["fc3T"] = np.concatenate([fc3T[128 * k:128 * (k + 1)] for k in range(2)], axis=1)
    w["eye9"] = (g("stn_fc3_b") + np.eye(3, dtype=np.float32).reshape(9)).reshape(9, 1)
    w1 = g("conv1_w")
    w1a, w1d = w1[:, :3], w1[:, 3:] - w1[:, :3]
    z1a = np.zeros((128, 4 * 128), np.float32)
    z1d = np.zeros((128, 4 * 128), np.float32)
    for pp in range(4):
        for i in range(3):
            z1a[16 * pp + i, 128 * pp:128 * pp + 64] = w1a[:, i]
            z1a[16 * (pp + 4) + i, 128 * pp + 64:128 * pp + 128] = w1a[:, i]
            z1d[16 * pp + i, 128 * pp:128 * pp + 64] = w1d[:, i]
            z1d[16 * (pp + 4) + i, 128 * pp + 64:128 * pp + 128] = w1d[:, i]
    w["z1a"], w["z1d"] = z1a, z1d
    diag2 = lambda m: np.block(
        [[m, np.zeros_like(m)], [np.zeros_like(m), m]]).astype(np.float32)
    w["d2w2"] = diag2(g("conv2_w").T)
    w3 = g("conv3_w")
    w["d2w3a"] = diag2(np.ascontiguousarray(w3[:, :64].T))
    w["d2w3d"] = diag2(np.ascontiguousarray((w3[:, 64:] - w3[:, :64]).T))
    w["d2w4"] = diag2(g("conv4_w").T)
    w5 = g("conv5_w")
    w["w5aT"] = np.ascontiguousarray(w5[:, :64].T)
    w5d = np.ascontiguousarray((w5[:, 64:] - w5[:, :64]).T)
    z5 = np.zeros((64, 256), np.float32)
    z5[:, 0:64] = w5d
    z5[:, 128:192] = w5d
    w["w5d2"] = z5
    w6T = g("conv6_w").T          # [192, 1024]
    w["w6k0"] = np.ascontiguousarray(w6T[0:64])
    w["w6k1"] = np.ascontiguousarray(w6T[64:128])
    w["w6k2"] = np.ascontiguousarray(w6T[128:192])
    w8 = g("conv8_w")
    w8gT = w8[:, :1024].T
    w["w8g"] = np.concatenate([w8gT[128 * k:128 * (k + 1)] for k in range(8)], axis=1)
    w["w8l"] = np.ascontiguousarray(w8[:, 1024:1088].T)
    w8xT = w8[:, 1088:].T          # [192, 256]
    w["w8x0"] = np.ascontiguousarray(w8xT[0:64])
    w["w8x1"] = np.ascontiguousarray(w8xT[64:128])
    w["w8x2"] = np.ascontiguousarray(w8xT[128:192])
    w9T = g("conv9_w").T
    w["w9T"] = np.concatenate([w9T[0:128], w9T[128:256]], axis=1)
    w10T = g("conv10_w").T
    w["w10T"] = np.concatenate([w10T[0:128], w10T[128:256]], axis=1)
    w["w11T"] = np.ascontiguousarray(g("conv11_w").T)
    w["neg1"] = np.full((64, 1), -1.0, np.float32)
    w["ones1"] = np.ones((1, 2048), np.float32)
    w = {k: np.ascontiguousarray(v, dtype=np.float32) for k, v in w.items()}
    return w


WSHAPES = dict(
    c1T=[3, 64], c2T=[64, 128], c3T=[128, 1024], fc1T=[128, 4096],
    fc2T=[128, 1024], fc3T=[128, 18], eye9=[9, 1],
    z1a=[128, 512], z1d=[128, 512], d2w2=[128, 128], d2w3a=[128, 128],
    d2w3d=[128, 128], d2w4=[128, 128], w5aT=[64, 64], w5d2=[64, 256],
    w6k0=[64, 1024], w6k1=[64, 1024], w6k2=[64, 1024],
    w8g=[128, 2048], w8l=[64, 256], w8x0=[64, 256], w8x1=[64, 256],
    w8x2=[64, 256], w9T=[128, 512], w10T=[128, 256], w11T=[128, 3],
    neg1=[64, 1], ones1=[1, 2048],
)


class _Ctx:
    pass


def _build():
    nc = bacc.Bacc(None, target_bir_lowering=False)
    C = _Ctx()
    C.nc = nc
    C.x_in = nc.dram_tensor("x", [BPC, 3, N], F32, kind="ExternalInput")
    C.lv_in = nc.dram_tensor("lv", [BPC, 64], F32, kind="ExternalInput")
    C.y_out = nc.dram_tensor("y", [BPC, 3, N], F32, kind="ExternalOutput")
    wd = {k: nc.dram_tensor("w_" + k, s, F32, kind="ExternalInput")
          for k, s in WSHAPES.items()}
    C.wd_ones = wd["ones1"]
    C.xcD = [nc.dram_tensor(f"xcD{b}", [3 * N], F32) for b in range(BPC)]
    C.idxD = [[nc.dram_tensor(f"idxD{b}_{g}", [N * K], I16) for g in range(3)]
              for b in range(BPC)]
    C.xD = [[nc.dram_tensor(f"xD{b}_{g}", [64 * N], F32) for g in range(2)]
            for b in range(BPC)]

    with tile.TileContext(nc) as tc:
        with (
            tc.tile_pool(name="wp", bufs=1) as wp,
            tc.tile_pool(name="fp", bufs=1) as fp,
            tc.tile_pool(name="dp", bufs=2) as dp,
            tc.tile_pool(name="dmr", bufs=1) as dmr,
            tc.tile_pool(name="ep", bufs=1) as ep,
            tc.tile_pool(name="gp", bufs=2) as gp,
            tc.tile_pool(name="sp", bufs=2) as sp,
            tc.tile_pool(name="psb", bufs=2, space="PSUM") as psb,
            tc.tile_pool(name="pfc", bufs=2, space="PSUM") as pfc,
            tc.tile_pool(name="pbig", bufs=1, space="PSUM") as pbig,
        ):
            C.fp, C.dp, C.dmr, C.ep, C.gp, C.sp = fp, dp, dmr, ep, gp, sp
            C.psb, C.pfc, C.pbig = psb, pfc, pbig
            C.W = {}
            for k, t in wd.items():
                if k == "ones1":
                    continue
                C.W[k] = wp.tile(WSHAPES[k], F32, tag="w_" + k, name="wt_" + k)
                nc.gpsimd.dma_start(C.W[k][:], t[:])
            for b in range(BPC):
                _cloud(C, b)
    nc.finalize()
    return nc


def wones(C):
    return C.wd_ones.ap()


def _act(nc, out, in_, func=AF.Identity, bias=0.0, scale=1.0):
    nc.scalar.activation(out, in_, func, bias=bias, scale=scale, alpha=0.0)


def _prelu(nc, out, in_, bias=0.0):
    nc.scalar.activation(out, in_, AF.Prelu, bias=bias, scale=1.0, alpha=0.2)


def _graph_prep(C, fe, Cc):
    """Cc==64: RB = [2F (0:64); -sq (64)]; fe rows 0:64 = F, row 64 = ones.
    (engine writes at partitions 0 and 64 only)"""
    nc = C.nc
    F2 = C.fp.tile([64, N], F32, tag="C")
    _act(nc, F2[0:Cc, :], fe[0:Cc, :], AF.Square)
    RB = C.fp.tile([65, N], F32, tag="RB")
    _act(nc, RB[0:Cc, :], fe[0:Cc, :], scale=2.0)
    for c in range(4):
        ps = C.psb.tile([128, 512], F32, tag="ps")
        nc.tensor.matmul(ps[0:1, :], C.W["neg1"][0:Cc, :],
                         F2[0:Cc, 512 * c:512 * (c + 1)], start=True, stop=True)
        _act(nc, RB[Cc:Cc + 1, 512 * c:512 * (c + 1)], ps[0:1, :])
    return RB


def _graph_prep1(C, xr3):
    """Graph-1: RB1 = [-sq (row0); xr (rows 1:4)] built with partition-0
    engine writes + DMA placement. Pairs with lhsT fe1s = [ones; 2*xr]."""
    nc = C.nc
    F2 = C.fp.tile([64, N], F32, tag="C")
    _act(nc, F2[0:3, :], xr3[:], AF.Square)
    RB = C.fp.tile([65, N], F32, tag="RB")
    for c in range(4):
        ps = C.psb.tile([128, 512], F32, tag="ps")
        nc.tensor.matmul(ps[0:1, :], C.W["neg1"][0:3, :],
                         F2[0:3, 512 * c:512 * (c + 1)], start=True, stop=True)
        _act(nc, RB[0:1, 512 * c:512 * (c + 1)], ps[0:1, :])
    nc.sync.dma_start(RB[1:4, :], xr3[:])
    return RB


def _dist_topk(C, fe, Cc, RB, idxDg):
    nc = C.nc
    for t in range(NT):
        d1 = C.dp.tile([128, N], F32, tag="dA")
        for c in range(4):
            ps = C.psb.tile([128, 512], F32, tag="ps")
            nc.tensor.matmul(ps[:], fe[0:Cc + 1, 128 * t:128 * (t + 1)],
                             RB[0:Cc + 1, 512 * c:512 * (c + 1)],
                             start=True, stop=True)
            _act(nc, d1[:, 512 * c:512 * (c + 1)], ps[:])
        idx24 = C.sp.tile([128, 24], U16, tag="idx24")
        v8a = C.sp.tile([128, 8], F32, tag="v8a")
        v8b = C.sp.tile([128, 8], F32, tag="v8b")
        v8c = C.sp.tile([128, 8], F32, tag="v8c")
        d2 = C.dmr.tile([128, N], F32, tag="dB")
        nc.vector.max(v8a[:], d1[:])
        nc.vector.max_index(idx24[:, 0:8], v8a[:], d1[:])
        nc.vector.match_replace(d2[:], v8a[:], d1[:], NEG)
        nc.vector.max(v8b[:], d2[:])
        nc.vector.max_index(idx24[:, 8:16], v8b[:], d2[:])
        nc.vector.match_replace(d1[:], v8b[:], d2[:], NEG)
        nc.vector.max(v8c[:], d1[:])
        nc.vector.max_index(idx24[:, 16:24], v8c[:], d1[:])
        nc.sync.dma_start(bass.AP(idxDg, 2560 * t, [[K, 128], [1, K]]),
                          idx24[:, 0:K].bitcast(I16))


def _conv_ctr_chunks(C, mm_a, mm_d, ctr, ctr_base, e1, qtotal):
    """Accumulating conv over gathered cols + broadcast ctr term.
    500-col chunks at 512-aligned psum offsets; strided ACT drain -> e1."""
    nc = C.nc
    done = 0
    while done < qtotal:
        grp = []
        for s in range(4):
            if done >= qtotal:
                break
            ln = min(500, qtotal - done)
            grp.append((done, 512 * s, ln))
            done += ln
        pb = C.pbig.tile([128, 2048], F32, tag="pb")
        for (qoff, poff, ln) in grp:
            mm_a(pb[:, poff:poff + ln], qoff, ln)
            npts = ln // K
            cap = ctr[:, ctr_base + qoff // K: ctr_base + qoff // K + npts]
            mm_d(pb[:, poff:poff + ln], cap.unsqueeze(-1).broadcast_to([128, npts, K]))
        if all(ln == 500 for (_, _, ln) in grp) and len(grp) > 1:
            ng = len(grp)
            v = pb[:, 0:512 * ng].rearrange("p (s c) -> p s c", c=512)[:, :, 0:500]
            dst = e1[:, grp[0][0]:grp[0][0] + 500 * ng].rearrange(
                "p (s c) -> p s c", c=500)
            _prelu(nc, dst, v)
        else:
            for (qoff, poff, ln) in grp:
                _prelu(nc, e1[:, qoff:qoff + ln], pb[:, poff:poff + ln])


def _conv2_maxk(C, w2, e1, qtotal, xh, xh_base):
    """Second conv (block-diag) + in-place Prelu + segmented max over K."""
    nc = C.nc
    for s in range(qtotal // 1280):
        pb = C.pbig.tile([128, 2048], F32, tag="pb")
        for (po, ln) in ((0, 512), (512, 512), (1024, 256)):
            nc.tensor.matmul(pb[:, po:po + ln], C.W[w2][:],
                             e1[:, 1280 * s + po:1280 * s + po + ln],
                             start=True, stop=True)
        e2 = C.dp.tile([128, 1280], F32, tag="dA")
        _prelu(nc, e2[:], pb[:, 0:1280])
        nc.vector.tensor_reduce(
            xh[:, xh_base + 64 * s:xh_base + 64 * (s + 1)],
            e2[:].rearrange("p (g k) -> p g k", k=K),
            op=ALU.max, axis=AX.X)


def _cloud(C, b):
    nc = C.nc
    fp, psb, pbig, dp = C.fp, C.psb, C.pbig, C.dp
    W = C.W
    xcDb, idxDb, xDb = C.xcD[b], C.idxD[b], C.xD[b]

    xc = fp.tile([3, N], F32, tag="C")
    nc.gpsimd.dma_start(xc[:], C.x_in.ap()[b])

    # ---- STN ----
    h1 = fp.tile([64, N], F32, tag="A")
    for c in range(4):
        ps = psb.tile([128, 512], F32, tag="ps")
        nc.tensor.matmul(ps[0:64, :], W["c1T"][:], xc[:, 512 * c:512 * (c + 1)],
                         start=True, stop=True)
        _act(nc, h1[:, 512 * c:512 * (c + 1)], ps[0:64, :], AF.Relu)
    h2 = fp.tile([128, N], F32, tag="B")
    for c in range(4):
        ps = psb.tile([128, 512], F32, tag="ps")
        nc.tensor.matmul(ps[:], W["c2T"][:], h1[:, 512 * c:512 * (c + 1)],
                         start=True, stop=True)
        _act(nc, h2[:, 512 * c:512 * (c + 1)], ps[:], AF.Relu)
    gcol = fp.tile([128, 8], F32, tag="gcol")
    for m in range(8):
        pb = pbig.tile([128, 2048], F32, tag="pb")
        for c in range(4):
            nc.tensor.matmul(pb[:, 512 * c:512 * (c + 1)],
                             W["c3T"][:, 128 * m:128 * (m + 1)],
                             h2[:, 512 * c:512 * (c + 1)], start=True, stop=True)
        nc.vector.tensor_reduce(gcol[:, m:m + 1], pb[:], op=ALU.max, axis=AX.X)
    g1 = fp.tile([128, 8], F32, tag="g1")
    _act(nc, g1[:], gcol[:], AF.Relu)
    ps1 = C.pfc.tile([128, 4], F32, tag="fc")
    for m in range(4):
        for k in range(8):
            nc.tensor.matmul(ps1[:, m:m + 1],
                             W["fc1T"][:, 512 * k + 128 * m:512 * k + 128 * (m + 1)],
                             g1[:, k:k + 1], start=(k == 0), stop=(k == 7))
    g2 = fp.tile([128, 4], F32, tag="g2")
    _act(nc, g2[:], ps1[:], AF.Relu)
    ps2 = C.pfc.tile([128, 2], F32, tag="fc")
    for m in range(2):
        for k in range(4):
            nc.tensor.matmul(ps2[:, m:m + 1],
                             W["fc2T"][:, 256 * k + 128 * m:256 * k + 128 * (m + 1)],
                             g2[:, k:k + 1], start=(k == 0), stop=(k == 3))
    g3 = fp.tile([128, 2], F32, tag="g3")
    _act(nc, g3[:], ps2[:], AF.Relu)
    ps3 = C.pfc.tile([9, 1], F32, tag="fc")
    for k in range(2):
        nc.tensor.matmul(ps3[:], W["fc3T"][:, 9 * k:9 * (k + 1)], g3[:, k:k + 1],
                         start=(k == 0), stop=(k == 1))
    t9 = fp.tile([9, 1], F32, tag="t9")
    nc.scalar.activation(t9[:], ps3[:], AF.Identity, bias=W["eye9"][:], scale=1.0)
    trans = fp.tile([3, 3], F32, tag="trans")
    for c in range(3):
        nc.sync.dma_start(trans[c:c + 1, 0:3], t9[3 * c:3 * c + 3, 0:1])

    # ---- rotate; fe1s = [ones ; 2*xr] ----
    xr3 = fp.tile([3, N], F32, tag="xr3")
    for c in range(4):
        ps = psb.tile([128, 512], F32, tag="ps")
        nc.tensor.matmul(ps[0:3, :], trans[:], xc[:, 512 * c:512 * (c + 1)],
                         start=True, stop=True)
        _act(nc, xr3[:, 512 * c:512 * (c + 1)], ps[0:3, :])
    x2r3 = fp.tile([3, N], F32, tag="C")
    _act(nc, x2r3[:], xr3[:], scale=2.0)
    fe1 = fp.tile([65, N], F32, tag="fe1")
    nc.sync.dma_start(fe1[0:1, :], wones(C))
    nc.sync.dma_start(fe1[1:4, :], x2r3[:])
    nc.sync.dma_start(bass.AP(xcDb, 0, [[N, 3], [1, N]]), xr3[:])

    if TRUNC == 1:
        nc.sync.dma_start(C.y_out.ap()[b], xr3[:])
        return

    # ================= graph 1 =================
    RB = _graph_prep1(C, xr3)
    _dist_topk(C, fe1, 3, RB, idxDb[0])
    if TRUNC == 2:
        nc.sync.dma_start(C.y_out.ap()[b], xr3[:])
        return

    xcrep = fp.tile([128, N], F32, tag="B")
    nc.gpsimd.memset(xcrep[:], 0.0)
    for c in range(8):
        nc.sync.dma_start(xcrep[16 * c:16 * c + 3, :],
                          bass.AP(xcDb, 0, [[N, 3], [1, N]]))
    xcctr = fp.tile([128, 256], F32, tag="ctr")
    nc.gpsimd.memset(xcctr[:], 0.0)
    for c in range(8):
        nc.sync.dma_start(xcctr[16 * c:16 * c + 3, :],
                          bass.AP(xcDb, 256 * c, [[N, 3], [1, 256]]))
    idx1 = fp.tile([128, 320], I16, tag="idxw1")
    for c in range(8):
        nc.sync.dma_start(idx1[16 * c:16 * (c + 1), :],
                          bass.AP(idxDb[0], 5120 * c, [[1, 16], [16, 320]]))
    xh1 = fp.tile([128, 1024], F32, tag="xh")
    for h in range(2):
        g1t = C.gp.tile([128, 2560], F32, tag="gout")
        nc.gpsimd.ap_gather(g1t[:], xcrep[:], idx1[:, 160 * h:160 * (h + 1)],
                            channels=128, num_elems=N, d=1, num_idxs=2560)
        for pp in range(4):
            e1 = C.ep.tile([128, 2560], F32, tag="e1")
            wa, wdd = W["z1a"], W["z1d"]

            def mm_a(dst, qoff, ln, _g=g1t, _pp=pp):
                nc.tensor.matmul(dst, wa[:, 128 * _pp:128 * (_pp + 1)],
                                 _g[:, qoff:qoff + ln], start=True, stop=False)

            def mm_d(dst, cap, _pp=pp):
                nc.tensor.matmul(dst, wdd[:, 128 * _pp:128 * (_pp + 1)],
                                 cap, start=False, stop=True)

            _conv_ctr_chunks(C, mm_a, mm_d, xcctr, 128 * h, e1, 2560)
            _conv2_maxk(C, "d2w2", e1, 2560, xh1, 256 * pp + 128 * h)
    if TRUNC == 3:
        nc.sync.dma_start(C.y_out.ap()[b], xr3[:])
        return
    fe2 = fp.tile([65, N], F32, tag="fe2")
    nc.sync.dma_start(fe2[0:64, 0:1024], xh1[0:64, :])
    nc.sync.dma_start(fe2[0:64, 1024:2048], xh1[64:128, :])
    nc.sync.dma_start(fe2[64:65, :], wones(C))

    # ================= graph 2 =================
    RB = _graph_prep(C, fe2, 64)
    _dist_topk(C, fe2, 64, RB, idxDb[1])
    nc.sync.dma_start(bass.AP(xDb[0], 0, [[N, 64], [1, N]]), fe2[0:64, :])
    dup = fp.tile([128, N], F32, tag="B")
    for h in range(2):
        nc.sync.dma_start(dup[64 * h:64 * (h + 1), :],
                          bass.AP(xDb[0], 0, [[N, 64], [1, N]]))
    ctr = fp.tile([128, 1024], F32, tag="ctr")
    for h in range(2):
        nc.sync.dma_start(ctr[64 * h:64 * (h + 1), :],
                          bass.AP(xDb[0], 1024 * h, [[N, 64], [1, 1024]]))
    idxw = fp.tile([128, 1280], I16, tag="idxw2")
    for c in range(8):
        nc.sync.dma_start(idxw[16 * c:16 * (c + 1), :],
                          bass.AP(idxDb[1], 20480 * (c // 4), [[1, 16], [16, 1280]]))
    xh2 = fp.tile([128, 1024], F32, tag="xh")
    for t in range(8):
        gt = C.gp.tile([128, 2560], F32, tag="gout")
        nc.gpsimd.ap_gather(gt[:], dup[:], idxw[:, 160 * t:160 * (t + 1)],
                            channels=128, num_elems=N, d=1, num_idxs=2560)
        e1 = C.ep.tile([128, 2560], F32, tag="e1")

        def mm_a3(dst, qoff, ln, _g=gt):
            nc.tensor.matmul(dst, W["d2w3a"][:], _g[:, qoff:qoff + ln],
                             start=True, stop=False)

        def mm_d3(dst, cap):
            nc.tensor.matmul(dst, W["d2w3d"][:], cap, start=False, stop=True)

        _conv_ctr_chunks(C, mm_a3, mm_d3, ctr, 128 * t, e1, 2560)
        _conv2_maxk(C, "d2w4", e1, 2560, xh2, 128 * t)
    fe3 = fp.tile([65, N], F32, tag="fe3")
    nc.sync.dma_start(fe3[0:64, 0:1024], xh2[0:64, :])
    nc.sync.dma_start(fe3[0:64, 1024:2048], xh2[64:128, :])
    nc.sync.dma_start(fe3[64:65, :], wones(C))

    # ================= graph 3 =================
    RB = _graph_prep(C, fe3, 64)
    _dist_topk(C, fe3, 64, RB, idxDb[2])
    a5 = fp.tile([64, N], F32, tag="A")
    for c in range(4):
        ps = psb.tile([128, 512], F32, tag="ps")
        nc.tensor.matmul(ps[0:64, :], W["w5aT"][:],
                         fe3[0:64, 512 * c:512 * (c + 1)], start=True, stop=True)
        _act(nc, a5[:, 512 * c:512 * (c + 1)], ps[0:64, :])
    nc.sync.dma_start(bass.AP(xDb[1], 0, [[N, 64], [1, N]]), a5[:])
    dup3 = fp.tile([128, N], F32, tag="B")
    for h in range(2):
        nc.sync.dma_start(dup3[64 * h:64 * (h + 1), :],
                          bass.AP(xDb[1], 0, [[N, 64], [1, N]]))
    idxw3 = fp.tile([128, 1280], I16, tag="idxw2")
    for c in range(8):
        nc.sync.dma_start(idxw3[16 * c:16 * (c + 1), :],
                          bass.AP(idxDb[2], 20480 * (c // 4), [[1, 16], [16, 1280]]))
    xh3 = fp.tile([128, 1024], F32, tag="xh")
    for t in range(8):
        gt = C.gp.tile([128, 2560], F32, tag="gout")
        nc.gpsimd.ap_gather(gt[:], dup3[:], idxw3[:, 160 * t:160 * (t + 1)],
                            channels=128, num_elems=N, d=1, num_idxs=2560)
        nc.vector.tensor_reduce(xh3[:, 128 * t:128 * (t + 1)],
                                gt[:].rearrange("p (g k) -> p g k", k=K),
                                op=ALU.max, axis=AX.X)
    pb5 = pbig.tile([128, 2048], F32, tag="pb")
    for h in range(2):
        for c in range(2):
            nc.tensor.matmul(pb5[64 * h:64 * (h + 1), 512 * c:512 * (c + 1)],
                             W["w5d2"][:, 128 * h:128 * h + 64],
                             fe3[0:64, 1024 * h + 512 * c:1024 * h + 512 * (c + 1)],
                             start=True, stop=True)
    x3h = fp.tile([128, 1024], F32, tag="x3h")
    nc.vector.tensor_tensor(out=x3h[:], in0=xh3[:], in1=pb5[:, 0:1024], op=ALU.add)
    _prelu(nc, x3h[:], x3h[:])
    x3t = fp.tile([64, N], F32, tag="x3t")
    nc.sync.dma_start(x3t[:, 0:1024], x3h[0:64, :])
    nc.sync.dma_start(x3t[:, 1024:2048], x3h[64:128, :])

    # ---- conv6 + global max ----
    g6 = fp.tile([128, 8], F32, tag="g6")
    for m in range(8):
        pb = pbig.tile([128, 2048], F32, tag="pb")
        for c in range(4):
            sl = slice(512 * c, 512 * (c + 1))
            nc.tensor.matmul(pb[:, sl], W["w6k0"][:, 128 * m:128 * (m + 1)],
                             fe2[0:64, sl], start=True, stop=False)
            nc.tensor.matmul(pb[:, sl], W["w6k1"][:, 128 * m:128 * (m + 1)],
                             fe3[0:64, sl], start=False, stop=False)
            nc.tensor.matmul(pb[:, sl], W["w6k2"][:, 128 * m:128 * (m + 1)],
                             x3t[:, sl], start=False, stop=True)
        nc.vector.tensor_reduce(g6[:, m:m + 1], pb[:], op=ALU.max, axis=AX.X)
    g6a = fp.tile([128, 8], F32, tag="g6a")
    _prelu(nc, g6a[:], g6[:])
    lv = fp.tile([64, 1], F32, tag="lv")
    nc.gpsimd.dma_start(lv[:], C.lv_in.ap()[b].unsqueeze(-1))
    ps8 = C.pfc.tile([128, 2], F32, tag="fc")
    for m in range(2):
        for k in range(8):
            nc.tensor.matmul(ps8[:, m:m + 1],
                             W["w8g"][:, 256 * k + 128 * m:256 * k + 128 * (m + 1)],
                             g6a[:, k:k + 1], start=(k == 0), stop=False)
        nc.tensor.matmul(ps8[:, m:m + 1], W["w8l"][:, 128 * m:128 * (m + 1)],
                         lv[:], start=False, stop=True)
    g8 = fp.tile([128, 2], F32, tag="g8")
    _act(nc, g8[:], ps8[:])

    h8a = fp.tile([128, N], F32, tag="B")
    h8b = C.dmr.tile([128, N], F32, tag="dB")
    h8t = [h8a, h8b]
    for m in range(2):
        for c in range(4):
            sl = slice(512 * c, 512 * (c + 1))
            ps = psb.tile([128, 512], F32, tag="ps")
            nc.tensor.matmul(ps[:], W["w8x0"][:, 128 * m:128 * (m + 1)],
                             fe2[0:64, sl], start=True, stop=False)
            nc.tensor.matmul(ps[:], W["w8x1"][:, 128 * m:128 * (m + 1)],
                             fe3[0:64, sl], start=False, stop=False)
            nc.tensor.matmul(ps[:], W["w8x2"][:, 128 * m:128 * (m + 1)],
                             x3t[:, sl], start=False, stop=True)
            _prelu(nc, h8t[m][:, sl], ps[:], bias=g8[:, m:m + 1])
    h9a = dp.tile([128, N], F32, tag="dA")
    h9b = dp.tile([128, N], F32, tag="dA")
    h9t = [h9a, h9b]
    for m in range(2):
        for c in range(4):
            sl = slice(512 * c, 512 * (c + 1))
            ps = psb.tile([128, 512], F32, tag="ps")
            for k in range(2):
                nc.tensor.matmul(ps[:],
                                 W["w9T"][:, 256 * k + 128 * m:256 * k + 128 * (m + 1)],
                                 h8t[k][:, sl], start=(k == 0), stop=(k == 1))
            _prelu(nc, h9t[m][:, sl], ps[:])
    h10 = fp.tile([128, N], F32, tag="A")
    for c in range(4):
        sl = slice(512 * c, 512 * (c + 1))
        ps = psb.tile([128, 512], F32, tag="ps")
        for k in range(2):
            nc.tensor.matmul(ps[:], W["w10T"][:, 128 * k:128 * (k + 1)],
                             h9t[k][:, sl], start=(k == 0), stop=(k == 1))
        _prelu(nc, h10[:, sl], ps[:])
    o3 = fp.tile([3, N], F32, tag="C")
    for c in range(4):
        ps = psb.tile([128, 512], F32, tag="ps")
        nc.tensor.matmul(ps[0:3, :], W["w11T"][:], h10[:, 512 * c:512 * (c + 1)],
                         start=True, stop=True)
        nc.scalar.activation(o3[:, 512 * c:512 * (c + 1)], ps[0:3, :], AF.Tanh)
    nc.sync.dma_start(C.y_out.ap()[b], o3[:])


def kernel(x, l, params):
    x = np.asarray(x, np.float32)
    l = np.asarray(l)
    if "nc" not in _cache:
        _cache["nc"] = _build()
    nc = _cache["nc"]
    w = _prep_weights(params)
    w7 = np.asarray(params["conv7_w"], np.float32)
    in_maps = []
    for core in range(8):
        m = {"x": np.ascontiguousarray(x[2 * core:2 * core + 2].transpose(0, 2, 1))}
        lv = np.stack([_lrelu_np(w7[:, int(l[2 * core + bb])]) for bb in range(BPC)])
        m["lv"] = np.ascontiguousarray(lv, dtype=np.float32)
        for k in WSHAPES:
            m["w_" + k] = w[k]
        in_maps.append(m)
    res = run_bass_kernel_spmd(nc, in_maps, list(range(8)))
    out = np.concatenate([res.results[c]["y"] for c in range(8)], axis=0)
    return np.ascontiguousarray(out.transpose(0, 2, 1)).astype(np.float32)
